# revision 9
# baseline (speedup 1.0000x reference)
"""Causal multi-head attention (B=2, S=2048, D=1024, H=16, HD=64) on 8 trn2 cores.

Sharding: 2 heads per core x both batches (head-parallel QKV/attention/out-proj,
Wo h-split => per-core partial outputs, summed on host).

Per-core device program (fp32 throughout):
  - QT/KT [hhd=128, S] via matmul(lhsT=W d-chunk [128,128], rhs=XT [128,512])
  - V [s, hhd] with an extra ones column (V_aug) for softmax row sums
  - scores^T chunks [k=128, q=512] = lhsT=KT_h [64,128] @ rhs=QT_h [64,512]
  - exp on ScalarE (scale=1/8 fused); causal zeroing via gpsimd.affine_select
  - attnV: o_psum [65, 512] += lhsT=V_aug [128,65] @ rhs=E [128,512]
    (row 64 accumulates the softmax denominator)
  - normalize: DVE reciprocal of row sums, PE broadcast (ones[1,64] outer rinv),
    DVE multiply
  - out-proj: lhsT=OTn [128,128-s-chunk] @ rhs=Wo [128,512], +bias, DMA out

The full causal mask is honored by only computing k-chunks at or below the
diagonal and zeroing the partial diagonal groups.
"""

import numpy as np

import concourse.bass as bass
import concourse.mybir as mybir
import concourse.tile as tile
from concourse import bacc
from concourse.bass_utils import run_bass_kernel_spmd

F32 = mybir.dt.float32
AF = mybir.ActivationFunctionType

B, S, D, H, HD = 2, 2048, 1024, 16, 64
NCORES = 8
HPC = H // NCORES          # heads per core = 2
HH = HPC * HD              # 128 concat head dims per core
P = 128
DC = D // P                # 8 d-chunks
NQ = 512                   # q tile (psum bank width fp32)
QJ = S // NQ               # 4 q tiles
KC = S // P                # 16 k chunks
GK = 2                     # k-chunks per score group (psum: [128, GK, NQ])

_NC_CACHE = {}


def _build_nc(with_bias_qkv: bool, with_bias_o: bool, causal: bool):
    key = (with_bias_qkv, with_bias_o, causal)
    if key in _NC_CACHE:
        return _NC_CACHE[key]

    nc = bacc.Bacc("TRN2", target_bir_lowering=False, debug=False)
    xt = nc.dram_tensor("xt", [B, D, S], F32, kind="ExternalInput")
    wq = nc.dram_tensor("wq", [D, HH], F32, kind="ExternalInput")
    wk = nc.dram_tensor("wk", [D, HH], F32, kind="ExternalInput")
    wv = nc.dram_tensor("wv", [D, HH], F32, kind="ExternalInput")
    wo = nc.dram_tensor("wo", [HH, D], F32, kind="ExternalInput")
    if with_bias_qkv:
        bqkv = nc.dram_tensor("bqkv", [3, HH], F32, kind="ExternalInput")
    if with_bias_o:
        bo8 = nc.dram_tensor("bo8", [D], F32, kind="ExternalInput")
    out = nc.dram_tensor("out", [B, S, D], F32, kind="ExternalOutput")

    with tile.TileContext(nc) as tc:
        with (
            tc.tile_pool(name="const", bufs=1) as cpool,
            tc.tile_pool(name="xtp", bufs=1) as xt_pool,
            tc.tile_pool(name="qkv", bufs=2) as qkv_pool,
            tc.tile_pool(name="otp", bufs=1) as ot_pool,
            tc.tile_pool(name="ep", bufs=3) as e_pool,
            tc.tile_pool(name="osb", bufs=3) as osb_pool,
            tc.tile_pool(name="ps1", bufs=2, space="PSUM") as ps1,
            tc.tile_pool(name="ps2", bufs=1, space="PSUM") as ps2,
            tc.tile_pool(name="ps3", bufs=2, space="PSUM") as ps3,
        ):
            # ---- constants ----
            wq_sb = cpool.tile([P, DC, HH], F32, tag="wq", name="wq_sb")
            wk_sb = cpool.tile([P, DC, HH], F32, tag="wk", name="wk_sb")
            wv_sb = cpool.tile([P, DC, HH], F32, tag="wv", name="wv_sb")
            wo_sb = cpool.tile([P, D], F32, tag="wo", name="wo_sb")
            nc.sync.dma_start(wq_sb[:], wq.rearrange("(o p) f -> p o f", p=P))
            nc.sync.dma_start(wk_sb[:], wk.rearrange("(o p) f -> p o f", p=P))
            nc.sync.dma_start(wv_sb[:], wv.rearrange("(o p) f -> p o f", p=P))
            nc.sync.dma_start(wo_sb[:], wo[:])
            ones_sb = cpool.tile([P, NQ], F32, tag="ones", name="ones_sb")
            nc.vector.memset(ones_sb[:], 1.0)
            if with_bias_qkv:
                # rows at 32-aligned partitions (engine AP base must be 32-aligned)
                bqkv_sb = cpool.tile([65, HH], F32, tag="bqkv", name="bqkv_sb")
                for i in range(3):
                    nc.sync.dma_start(bqkv_sb[32 * i:32 * i + 1, :], bqkv[i:i + 1, :])
            if with_bias_o:
                bo8_sb = cpool.tile([1, D], F32, tag="bo8", name="bo8_sb")
                nc.sync.dma_start(bo8_sb[:], bo8.rearrange("(a d) -> a d", a=1))

            for b in range(B):
                # ---- load X^T for this batch ----
                xt_sb = xt_pool.tile([P, DC, S], F32, tag="xt", name="xt_sb")
                for d in range(DC):
                    nc.sync.dma_start(xt_sb[:, d, :], xt[b, d * P:(d + 1) * P, :])

                # ---- Q^T / K^T projections: [hhd=128, QJ, NQ] ----
                qt_sb = qkv_pool.tile([P, QJ, NQ], F32, tag="qt", name="qt_sb")
                kt_sb = qkv_pool.tile([P, QJ, NQ], F32, tag="kt", name="kt_sb")
                for w_sb, dst, brow in ((wq_sb, qt_sb, 0), (wk_sb, kt_sb, 1)):
                    for half in range(2):
                        pps = ps1.tile([P, 2, NQ], F32, tag="acc", name="proj_ps")
                        for j2 in range(2):
                            qj = half * 2 + j2
                            if with_bias_qkv:
                                nc.tensor.matmul(
                                    pps[:, j2, :],
                                    lhsT=bqkv_sb[32 * brow:32 * brow + 1, :],
                                    rhs=ones_sb[32 * brow:32 * brow + 1, :],
                                    start=True, stop=False,
                                )
                            for d in range(DC):
                                nc.tensor.matmul(
                                    pps[:, j2, :],
                                    lhsT=w_sb[:, d, :],
                                    rhs=xt_sb[:, d, qj * NQ:(qj + 1) * NQ],
                                    start=(d == 0 and not with_bias_qkv),
                                    stop=(d == DC - 1),
                                )
                        nc.scalar.copy(dst[:, half * 2:(half + 1) * 2, :], pps[:])

                # ---- V projection into [s, KC, HPC, HD+1] (V_aug) ----
                v_sb = qkv_pool.tile([P, KC, HPC, HD + 1], F32, tag="v", name="v_sb")
                nc.vector.memset(v_sb[:, :, :, HD:], 1.0)
                for t in range(2):
                    vps = ps1.tile([P, 2, NQ], F32, tag="acc", name="v_ps")
                    for sc8 in range(8):
                        sub = vps[:, sc8 // 4, (sc8 % 4) * P:(sc8 % 4 + 1) * P]
                        sc = t * 8 + sc8
                        if with_bias_qkv:
                            nc.tensor.matmul(
                                sub, lhsT=ones_sb[64:65, :P], rhs=bqkv_sb[64:65, :],
                                start=True, stop=False,
                            )
                        for d in range(DC):
                            nc.tensor.matmul(
                                sub,
                                lhsT=xt_sb[:, d, sc * P:(sc + 1) * P],
                                rhs=wv_sb[:, d, :],
                                start=(d == 0 and not with_bias_qkv),
                                stop=(d == DC - 1),
                            )
                    for sc8 in range(8):
                        sc = t * 8 + sc8
                        for h in range(HPC):
                            nc.vector.tensor_copy(
                                v_sb[:, sc, h, :HD],
                                vps[:, sc8 // 4, (sc8 % 4) * P + h * HD:(sc8 % 4) * P + (h + 1) * HD],
                            )

                # ---- attention per head ----
                ot_sb = ot_pool.tile([P, QJ, NQ], F32, tag="ot", name="ot_sb")
                # softmax denominators: row 32*(r%3), free slot r//3, r = h*QJ+qj
                rs_sb = ot_pool.tile([P, 3, NQ], F32, tag="rs", name="rs_sb")
                nc.vector.memset(rs_sb[:], 1.0)
                for h in range(HPC):
                    h0 = h * HD
                    for qj in range(QJ):
                        ngroups = (2 * (qj + 1)) if causal else (KC // GK)
                        o_ps = ps3.tile([HD + 1, NQ], F32, tag="o", name="o_ps")
                        for g in range(ngroups):
                            st_ps = ps1.tile([P, GK, NQ], F32, tag="acc", name="st_ps")
                            for c2 in range(GK):
                                ki = g * GK + c2
                                nc.tensor.matmul(
                                    st_ps[:, c2, :],
                                    lhsT=kt_sb[h0:h0 + HD, ki // 4, (ki % 4) * P:(ki % 4 + 1) * P],
                                    rhs=qt_sb[h0:h0 + HD, qj, :],
                                    start=True, stop=True,
                                )
                            e_sb = e_pool.tile([P, GK, NQ], F32, tag="e", name="e_sb")
                            nc.scalar.activation(e_sb[:], st_ps[:], AF.Exp, scale=0.125)
                            if causal and g >= 2 * qj:
                                # zero entries with k > q
                                nc.gpsimd.affine_select(
                                    out=e_sb[:],
                                    in_=e_sb[:],
                                    compare_op=mybir.AluOpType.is_ge,
                                    fill=0.0,
                                    base=qj * NQ - g * GK * P,
                                    pattern=[[-P, GK], [1, NQ]],
                                    channel_multiplier=-1,
                                )
                            for c2 in range(GK):
                                ki = g * GK + c2
                                nc.tensor.matmul(
                                    o_ps[:],
                                    lhsT=v_sb[:, ki, h, :],
                                    rhs=e_sb[:, c2, :],
                                    start=(g == 0 and c2 == 0),
                                    stop=(g == ngroups - 1 and c2 == GK - 1),
                                )
                        nc.vector.tensor_copy(ot_sb[h0:h0 + HD, qj, :], o_ps[:HD, :])
                        r = h * QJ + qj
                        nc.vector.tensor_copy(
                            rs_sb[32 * (r % 3):32 * (r % 3) + 1, r // 3, :],
                            o_ps[HD:HD + 1, :],
                        )

                # ---- normalization ----
                rinv_sb = ot_pool.tile([P, 3, NQ], F32, tag="rinv", name="rinv_sb")
                rsc_sb = ot_pool.tile([P, 3, NQ], F32, tag="rscr", name="rsc_sb")
                otn_sb = ot_pool.tile([P, QJ, NQ], F32, tag="otn", name="otn_sb")
                nc.vector.reciprocal_approx_accurate(
                    rinv_sb[:], rs_sb[:], scratch=rsc_sb[:]
                )
                for h in range(HPC):
                    h0 = h * HD
                    for qj in range(QJ):
                        r = h * QJ + qj
                        p0 = 32 * (r % 3)
                        bc_ps = ps3.tile([P, NQ], F32, tag="o", name="bc_ps")
                        nc.tensor.matmul(
                            bc_ps[h0:h0 + HD, :],
                            lhsT=ones_sb[p0:p0 + 1, :HD],
                            rhs=rinv_sb[p0:p0 + 1, r // 3, :],
                            start=True, stop=True,
                        )
                        nc.vector.tensor_mul(
                            otn_sb[h0:h0 + HD, qj, :],
                            ot_sb[h0:h0 + HD, qj, :],
                            bc_ps[h0:h0 + HD, :],
                        )

                # ---- output projection + store ----
                for sc in range(KC):
                    ops = ps2.tile([P, 2, NQ], F32, tag="op", name="op_ps")
                    for fc in range(2):
                        if with_bias_o:
                            nc.tensor.matmul(
                                ops[:, fc, :], lhsT=ones_sb[0:1, :P],
                                rhs=bo8_sb[:, fc * NQ:(fc + 1) * NQ],
                                start=True, stop=False,
                            )
                        nc.tensor.matmul(
                            ops[:, fc, :],
                            lhsT=otn_sb[:, sc // 4, (sc % 4) * P:(sc % 4 + 1) * P],
                            rhs=wo_sb[:, fc * NQ:(fc + 1) * NQ],
                            start=not with_bias_o, stop=True,
                        )
                    out_sb = osb_pool.tile([P, 2, NQ], F32, tag="out", name="out_sb")
                    if sc % 2 == 0:
                        nc.scalar.copy(out_sb[:], ops[:])
                    else:
                        nc.vector.tensor_copy(out_sb[:], ops[:])
                    nc.sync.dma_start(
                        out[b, sc * P:(sc + 1) * P, :],
                        out_sb.rearrange("p a n -> p (a n)"),
                    )

    nc.compile()
    _NC_CACHE[key] = nc
    return nc


def _check_causal(mask: np.ndarray) -> bool:
    m = np.asarray(mask).reshape(mask.shape[-2], mask.shape[-1])
    s = m.shape[0]
    if np.array_equal(m, np.tril(np.ones((s, s), dtype=bool))):
        return True
    if m.all():
        return False
    raise NotImplementedError("only causal or all-true masks are supported")


def kernel(inputs_q, mask, Wq, bq, Wk, bk, Wv, bv, Wo, bo, _trace=False,
           _trace_cores=None):
    inputs_q = np.asarray(inputs_q, dtype=np.float32)
    Wq = np.asarray(Wq, dtype=np.float32).reshape(D, H * HD)
    Wk = np.asarray(Wk, dtype=np.float32).reshape(D, H * HD)
    Wv = np.asarray(Wv, dtype=np.float32).reshape(D, H * HD)
    Wo = np.asarray(Wo, dtype=np.float32).reshape(H * HD, D)
    bq = np.asarray(bq, dtype=np.float32).reshape(H * HD)
    bk = np.asarray(bk, dtype=np.float32).reshape(H * HD)
    bv = np.asarray(bv, dtype=np.float32).reshape(H * HD)
    bo = np.asarray(bo, dtype=np.float32).reshape(D)

    causal = _check_causal(mask)
    with_bias_qkv = bool(bq.any() or bk.any() or bv.any())
    with_bias_o = bool(bo.any())

    nc = _build_nc(with_bias_qkv, with_bias_o, causal)

    xt = np.ascontiguousarray(inputs_q.transpose(0, 2, 1))  # [B, D, S]
    in_maps = []
    for c in range(NCORES):
        f0, f1 = c * HH, (c + 1) * HH
        m = {
            "xt": xt,
            "wq": np.ascontiguousarray(Wq[:, f0:f1]),
            "wk": np.ascontiguousarray(Wk[:, f0:f1]),
            "wv": np.ascontiguousarray(Wv[:, f0:f1]),
            "wo": np.ascontiguousarray(Wo[f0:f1, :]),
        }
        if with_bias_qkv:
            m["bqkv"] = np.ascontiguousarray(
                np.stack([bq[f0:f1], bk[f0:f1], bv[f0:f1]])
            )
        if with_bias_o:
            m["bo8"] = np.ascontiguousarray(bo / NCORES)
        in_maps.append(m)

    kwargs = {}
    if _trace:
        kwargs["trace"] = True
        if _trace_cores is not None:
            kwargs["trace_cores"] = _trace_cores
    res = run_bass_kernel_spmd(nc, in_maps, core_ids=list(range(NCORES)), **kwargs)

    acc = np.zeros((B, S, D), dtype=np.float64)
    for c in range(NCORES):
        acc += res.results[c]["out"]
    if not with_bias_o:
        acc += bo  # bo is zero here, but keep the math explicit
    out = acc.astype(np.float32)
    if _trace:
        return out, res
    return out


# revision 15
# speedup vs baseline: 1.9571x; 1.9571x over previous
"""Causal multi-head attention (B=2, S=2048, D=1024, H=16, HD=64) on 8 trn2 cores.

Sharding: 2 heads per core x both batches (head-parallel QKV/attention/out-proj,
Wo h-split => per-core partial outputs, summed on host).

All matmuls run in float32r (fp32 storage, reduced-precision single-pass PE
mode, ~1.4e-4 scale-relative matmul error vs 4-cycle/row full fp32); all
accumulation (PSUM), softmax and normalization stay fp32.

Per-core device program:
  - QT/KT/VT [hhd=128, S] via matmul(lhsT=W d-chunk [128,128], rhs=XT [128,512])
  - V [s, hhd(+ones col)] from VT via PE transpose (f32r N=128 matmuls are
    4-cycle/row, so the direct [s,hhd] projection is avoided)
  - scores^T chunks [k=128, q=512] = lhsT=KT_h [64,128] @ rhs=QT_h [64,512]
  - exp on ScalarE (scale=1/8 fused); causal zeroing via gpsimd.affine_select
  - attnV: o_psum [65, 512] += lhsT=V_aug [128,65] @ rhs=E [128,512]
    (row 64 accumulates the softmax denominator)
  - normalize: DVE reciprocal_approx of row sums, PE broadcast
    (ones[1,64] outer rinv), DVE multiply
  - out-proj: lhsT=OTn [128,128-s-chunk] @ rhs=Wo [128,512], +bias, DMA out

The causal mask is honored by only computing k-chunks at or below the
diagonal and zeroing the partial diagonal groups.
"""

import numpy as np

import concourse.bass as bass
import concourse.mybir as mybir
import concourse.tile as tile
from concourse import bacc
from concourse.masks import make_identity
from concourse.bass_utils import run_bass_kernel_spmd

F32 = mybir.dt.float32
F32R = mybir.dt.float32r
AF = mybir.ActivationFunctionType

B, S, D, H, HD = 2, 2048, 1024, 16, 64
NCORES = 8
HPC = H // NCORES          # heads per core = 2
HH = HPC * HD              # 128 concat head dims per core
P = 128
DC = D // P                # 8 d-chunks
NQ = 512                   # q tile (psum bank width fp32)
QJ = S // NQ               # 4 q tiles
KC = S // P                # 16 k chunks
GK = 2                     # k-chunks per score group (psum: [128, GK, NQ])

_NC_CACHE = {}


def _build_nc(with_bias_qkv: bool, with_bias_o: bool, causal: bool):
    key = (with_bias_qkv, with_bias_o, causal)
    if key in _NC_CACHE:
        return _NC_CACHE[key]

    nc = bacc.Bacc("TRN2", target_bir_lowering=False, debug=False)
    xt = nc.dram_tensor("xt", [B, D, S], F32R, kind="ExternalInput")
    wq = nc.dram_tensor("wq", [D, HH], F32R, kind="ExternalInput")
    wk = nc.dram_tensor("wk", [D, HH], F32R, kind="ExternalInput")
    wv = nc.dram_tensor("wv", [D, HH], F32R, kind="ExternalInput")
    wo = nc.dram_tensor("wo", [HH, D], F32R, kind="ExternalInput")
    if with_bias_qkv:
        bqkv = nc.dram_tensor("bqkv", [3, HH], F32, kind="ExternalInput")
    if with_bias_o:
        bo8 = nc.dram_tensor("bo8", [D], F32R, kind="ExternalInput")
    out = nc.dram_tensor("out", [B, S, D], F32, kind="ExternalOutput")

    with tile.TileContext(nc) as tc:
        with (
            tc.tile_pool(name="const", bufs=1) as cpool,
            tc.tile_pool(name="xtp", bufs=1) as xt_pool,
            tc.tile_pool(name="qkv", bufs=2) as qkv_pool,
            tc.tile_pool(name="otp", bufs=1) as ot_pool,
            tc.tile_pool(name="ep", bufs=3) as e_pool,
            tc.tile_pool(name="osb", bufs=2) as osb_pool,
            tc.tile_pool(name="ps1", bufs=2, space="PSUM") as ps1,
            tc.tile_pool(name="ps2", bufs=1, space="PSUM") as ps2,
            tc.tile_pool(name="ps3", bufs=2, space="PSUM") as ps3,
        ):
            # ---- constants ----
            wq_sb = cpool.tile([P, DC, HH], F32R, tag="wq", name="wq_sb")
            wk_sb = cpool.tile([P, DC, HH], F32R, tag="wk", name="wk_sb")
            wv_sb = cpool.tile([P, DC, HH], F32R, tag="wv", name="wv_sb")
            wo_sb = cpool.tile([P, D], F32R, tag="wo", name="wo_sb")
            nc.sync.dma_start(wq_sb[:], wq.rearrange("(o p) f -> p o f", p=P))
            nc.sync.dma_start(wk_sb[:], wk.rearrange("(o p) f -> p o f", p=P))
            nc.sync.dma_start(wv_sb[:], wv.rearrange("(o p) f -> p o f", p=P))
            nc.sync.dma_start(wo_sb[:], wo[:])
            ones_sb = cpool.tile([P, P], F32R, tag="ones", name="ones_sb")
            nc.vector.memset(ones_sb[:].bitcast(F32), 1.0)
            ident_sb = cpool.tile([P, P], F32, tag="ident", name="ident_sb")
            make_identity(nc, ident_sb[:])
            if with_bias_qkv:
                # per-partition bias columns: [:, 0]=bq, [:, 1]=bk, [:, 2]=bv
                bqkvt_sb = cpool.tile([HH, 3], F32, tag="bqkvt", name="bqkvt_sb")
                for i in range(3):
                    nc.sync.dma_start(
                        bqkvt_sb[:, i:i + 1], bqkv[i:i + 1, :].rearrange("a f -> f a")
                    )
            if with_bias_o:
                bo8_sb = cpool.tile([1, D], F32R, tag="bo8", name="bo8_sb")
                nc.sync.dma_start(bo8_sb[:], bo8.rearrange("(a d) -> a d", a=1))

            for b in range(B):
                # ---- load X^T for this batch ----
                xt_sb = xt_pool.tile([P, DC, S], F32R, tag="xt", name="xt_sb")
                for d in range(DC):
                    nc.sync.dma_start(xt_sb[:, d, :], xt[b, d * P:(d + 1) * P, :])

                # ---- Q^T / K^T / V^T projections: [hhd=128, QJ, NQ] ----
                qt_sb = qkv_pool.tile([P, QJ, NQ], F32R, tag="qt", name="qt_sb")
                kt_sb = qkv_pool.tile([P, QJ, NQ], F32R, tag="kt", name="kt_sb")
                # fp32 (f32r transpose-mode crashes the device; fp32 is the
                # production-tested PE-transpose path)
                vt_sb = qkv_pool.tile([P, QJ, NQ], F32, tag="vt", name="vt_sb",
                                      bufs=1)
                for w_sb, dst, bcol in ((wq_sb, qt_sb, 0), (wk_sb, kt_sb, 1),
                                        (wv_sb, vt_sb, 2)):
                    for half in range(2):
                        pps = ps1.tile([P, 2, NQ], F32, tag="acc", name="proj_ps")
                        for j2 in range(2):
                            qj = half * 2 + j2
                            for d in range(DC):
                                nc.tensor.matmul(
                                    pps[:, j2, :],
                                    lhsT=w_sb[:, d, :],
                                    rhs=xt_sb[:, d, qj * NQ:(qj + 1) * NQ],
                                    start=(d == 0),
                                    stop=(d == DC - 1),
                                )
                        if with_bias_qkv:
                            nc.scalar.activation(
                                dst[:, half * 2:(half + 1) * 2, :], pps[:],
                                AF.Identity, bias=bqkvt_sb[:, bcol:bcol + 1],
                            )
                        else:
                            nc.scalar.copy(dst[:, half * 2:(half + 1) * 2, :], pps[:])

                # ---- V [s, KC, HPC, HD+1] (V_aug) from VT via PE transpose ----
                v_sb = qkv_pool.tile([P, KC, HPC, HD + 1], F32R, tag="v", name="v_sb")
                nc.vector.memset(v_sb[:, :, :, HD:].bitcast(F32), 1.0)
                for sc in range(KC):
                    tp = ps3.tile([P, P], F32, tag="o", name="tr_ps")
                    nc.tensor.transpose(
                        tp[:], vt_sb[:, sc // 4, (sc % 4) * P:(sc % 4 + 1) * P],
                        ident_sb[:],
                    )
                    for h in range(HPC):
                        nc.vector.tensor_copy(
                            v_sb[:, sc, h, :HD], tp[:, h * HD:(h + 1) * HD]
                        )

                # ---- attention per head ----
                ot_sb = ot_pool.tile([P, QJ, NQ], F32, tag="ot", name="ot_sb")
                # softmax denominators: row 32*(r%3), free slot r//3, r = h*QJ+qj
                rs_sb = ot_pool.tile([P, 3, NQ], F32, tag="rs", name="rs_sb")
                nc.vector.memset(rs_sb[:], 1.0)
                for h in range(HPC):
                    h0 = h * HD
                    for qj in range(QJ):
                        ngroups = (2 * (qj + 1)) if causal else (KC // GK)
                        o_ps = ps3.tile([HD + 1, NQ], F32, tag="o", name="o_ps")
                        for g in range(ngroups):
                            st_ps = ps1.tile([P, GK, NQ], F32, tag="acc", name="st_ps")
                            for c2 in range(GK):
                                ki = g * GK + c2
                                nc.tensor.matmul(
                                    st_ps[:, c2, :],
                                    lhsT=kt_sb[h0:h0 + HD, ki // 4, (ki % 4) * P:(ki % 4 + 1) * P],
                                    rhs=qt_sb[h0:h0 + HD, qj, :],
                                    start=True, stop=True,
                                )
                            e_sb = e_pool.tile([P, GK, NQ], F32R, tag="e", name="e_sb")
                            nc.scalar.activation(e_sb[:], st_ps[:], AF.Exp, scale=0.125)
                            if causal and g >= 2 * qj:
                                # zero entries with k > q
                                nc.gpsimd.affine_select(
                                    out=e_sb[:],
                                    in_=e_sb[:],
                                    compare_op=mybir.AluOpType.is_ge,
                                    fill=0.0,
                                    base=qj * NQ - g * GK * P,
                                    pattern=[[-P, GK], [1, NQ]],
                                    channel_multiplier=-1,
                                )
                            for c2 in range(GK):
                                ki = g * GK + c2
                                nc.tensor.matmul(
                                    o_ps[:],
                                    lhsT=v_sb[:, ki, h, :],
                                    rhs=e_sb[:, c2, :],
                                    start=(g == 0 and c2 == 0),
                                    stop=(g == ngroups - 1 and c2 == GK - 1),
                                )
                        nc.vector.tensor_copy(ot_sb[h0:h0 + HD, qj, :], o_ps[:HD, :])
                        r = h * QJ + qj
                        nc.vector.tensor_copy(
                            rs_sb[32 * (r % 3):32 * (r % 3) + 1, r // 3, :],
                            o_ps[HD:HD + 1, :],
                        )

                # ---- normalization ----
                rsc_sb = ot_pool.tile([P, 3, NQ], F32, tag="rscr", name="rsc_sb")
                rinr_sb = ot_pool.tile([P, 3, NQ], F32R, tag="rinr", name="rinr_sb")
                otn_sb = ot_pool.tile([P, QJ, NQ], F32R, tag="otn", name="otn_sb")
                # reciprocal_approx_accurate with the final NR pass writing
                # (rounded) f32r directly
                from concourse.dve_ops import RECIPROCAL_APPROX_NR
                nc.vector.reciprocal_approx_fast(out=rsc_sb[:], in_=rs_sb[:])
                nc.vector._custom_dve(
                    RECIPROCAL_APPROX_NR, out=rinr_sb[:], in0=rs_sb[:],
                    in1=rsc_sb[:], s0=2.0,
                )
                for h in range(HPC):
                    h0 = h * HD
                    for qj in range(QJ):
                        r = h * QJ + qj
                        p0 = 32 * (r % 3)
                        bc_ps = ps3.tile([HD, NQ], F32, tag="o", name="bc_ps")
                        nc.tensor.matmul(
                            bc_ps[:],
                            lhsT=ones_sb[p0:p0 + 1, :HD],
                            rhs=rinr_sb[p0:p0 + 1, r // 3, :],
                            start=True, stop=True,
                        )
                        nc.vector.tensor_mul(
                            otn_sb[h0:h0 + HD, qj, :],
                            ot_sb[h0:h0 + HD, qj, :],
                            bc_ps[:],
                        )

                # ---- output projection + store ----
                for sc in range(KC):
                    ops = ps2.tile([P, 2, NQ], F32, tag="op", name="op_ps")
                    for fc in range(2):
                        if with_bias_o:
                            nc.tensor.matmul(
                                ops[:, fc, :], lhsT=ones_sb[0:1, :P],
                                rhs=bo8_sb[:, fc * NQ:(fc + 1) * NQ],
                                start=True, stop=False,
                            )
                        nc.tensor.matmul(
                            ops[:, fc, :],
                            lhsT=otn_sb[:, sc // 4, (sc % 4) * P:(sc % 4 + 1) * P],
                            rhs=wo_sb[:, fc * NQ:(fc + 1) * NQ],
                            start=not with_bias_o, stop=True,
                        )
                    out_sb = osb_pool.tile([P, 2, NQ], F32, tag="out", name="out_sb")
                    if sc % 2 == 0:
                        nc.scalar.copy(out_sb[:], ops[:])
                    else:
                        nc.vector.tensor_copy(out_sb[:], ops[:])
                    nc.sync.dma_start(
                        out[b, sc * P:(sc + 1) * P, :],
                        out_sb.rearrange("p a n -> p (a n)"),
                    )

    nc.compile()
    _NC_CACHE[key] = nc
    return nc


def _check_causal(mask: np.ndarray) -> bool:
    m = np.asarray(mask).reshape(mask.shape[-2], mask.shape[-1])
    s = m.shape[0]
    if np.array_equal(m, np.tril(np.ones((s, s), dtype=bool))):
        return True
    if m.all():
        return False
    raise NotImplementedError("only causal or all-true masks are supported")


def kernel(inputs_q, mask, Wq, bq, Wk, bk, Wv, bv, Wo, bo, _trace=False,
           _trace_cores=None):
    inputs_q = np.asarray(inputs_q, dtype=np.float32)
    Wq = np.asarray(Wq, dtype=np.float32).reshape(D, H * HD)
    Wk = np.asarray(Wk, dtype=np.float32).reshape(D, H * HD)
    Wv = np.asarray(Wv, dtype=np.float32).reshape(D, H * HD)
    Wo = np.asarray(Wo, dtype=np.float32).reshape(H * HD, D)
    bq = np.asarray(bq, dtype=np.float32).reshape(H * HD)
    bk = np.asarray(bk, dtype=np.float32).reshape(H * HD)
    bv = np.asarray(bv, dtype=np.float32).reshape(H * HD)
    bo = np.asarray(bo, dtype=np.float32).reshape(D)

    causal = _check_causal(mask)
    with_bias_qkv = bool(bq.any() or bk.any() or bv.any())
    with_bias_o = bool(bo.any())

    nc = _build_nc(with_bias_qkv, with_bias_o, causal)

    xt = np.ascontiguousarray(inputs_q.transpose(0, 2, 1))  # [B, D, S]
    in_maps = []
    for c in range(NCORES):
        f0, f1 = c * HH, (c + 1) * HH
        m = {
            "xt": xt,
            "wq": np.ascontiguousarray(Wq[:, f0:f1]),
            "wk": np.ascontiguousarray(Wk[:, f0:f1]),
            "wv": np.ascontiguousarray(Wv[:, f0:f1]),
            "wo": np.ascontiguousarray(Wo[f0:f1, :]),
        }
        if with_bias_qkv:
            m["bqkv"] = np.ascontiguousarray(
                np.stack([bq[f0:f1], bk[f0:f1], bv[f0:f1]])
            )
        if with_bias_o:
            m["bo8"] = np.ascontiguousarray(bo / NCORES)
        in_maps.append(m)

    kwargs = {}
    if _trace:
        kwargs["trace"] = True
        if _trace_cores is not None:
            kwargs["trace_cores"] = _trace_cores
    res = run_bass_kernel_spmd(nc, in_maps, core_ids=list(range(NCORES)), **kwargs)

    acc = np.zeros((B, S, D), dtype=np.float64)
    for c in range(NCORES):
        acc += res.results[c]["out"]
    if not with_bias_o:
        acc += bo  # bo is zero here, but keep the math explicit
    out = acc.astype(np.float32)
    if _trace:
        return out, res
    return out


# revision 17
# speedup vs baseline: 2.2207x; 1.1347x over previous
"""Causal multi-head attention (B=2, S=2048, D=1024, H=16, HD=64) on 8 trn2 cores.

Sharding: 2 heads per core x both batches (head-parallel QKV/attention/out-proj,
Wo h-split => per-core partial outputs, summed on host).

All matmuls run in float32r (single-pass PE mode, ~1.4e-4 matmul error vs
4-cycle/row full fp32); accumulation (PSUM), softmax and normalization are fp32.

Structure is organized to keep TensorE densely busy (the PE clock-gate
re-throttles to 1.2 GHz if it sees idle windows):
  - projections emitted d-chunk-major so matmuls start as X^T chunks land
  - attention processes both heads interleaved group-by-group; the two K=64
    score matmuls sit at partition bases 0/64 so the PE runs them concurrently
    (row tiling)
  - normalization + output projection run per q-tile, right after the q-tile's
    attnV accumulation finishes (spreads PE/ACT/DVE/DMA work, shrinks the tail)
  - batch 1's projections/transposes are sprinkled into batch 0's attention
    stream to fill PE gaps left by the exp (ScalarE) dependency

PSUM (8 banks): ps1 = scores + batch-0 proj [128,2,512]x2 (4), ps2 = out-proj
+ batch-1 proj [128,2,512]x1 (2), ps3 = attnV o / V-transpose / norm-broadcast
[*,512]x2 (2).
"""

import numpy as np

import concourse.bass as bass
import concourse.mybir as mybir
import concourse.tile as tile
from concourse import bacc
from concourse.bass_utils import run_bass_kernel_spmd
from concourse.masks import make_identity
from concourse.dve_ops import RECIPROCAL_APPROX_NR

F32 = mybir.dt.float32
F32R = mybir.dt.float32r
AF = mybir.ActivationFunctionType

B, S, D, H, HD = 2, 2048, 1024, 16, 64
NCORES = 8
HPC = H // NCORES          # heads per core = 2
HH = HPC * HD              # 128 concat head dims per core
P = 128
DC = D // P                # 8 d-chunks
NQ = 512                   # q tile (psum bank width fp32)
QJ = S // NQ               # 4 q tiles
KC = S // P                # 16 k chunks
GK = 2                     # k-chunks per score group (psum: [128, GK, NQ])

_NC_CACHE = {}


def _build_nc(with_bias_qkv: bool, with_bias_o: bool, causal: bool):
    key = (with_bias_qkv, with_bias_o, causal)
    if key in _NC_CACHE:
        return _NC_CACHE[key]

    nc = bacc.Bacc("TRN2", target_bir_lowering=False, debug=False)
    xt = nc.dram_tensor("xt", [B, D, S], F32R, kind="ExternalInput")
    wq = nc.dram_tensor("wq", [D, HH], F32R, kind="ExternalInput")
    wk = nc.dram_tensor("wk", [D, HH], F32R, kind="ExternalInput")
    wv = nc.dram_tensor("wv", [D, HH], F32R, kind="ExternalInput")
    wo = nc.dram_tensor("wo", [HH, D], F32R, kind="ExternalInput")
    if with_bias_qkv:
        bqkv = nc.dram_tensor("bqkv", [3, HH], F32, kind="ExternalInput")
    if with_bias_o:
        bo8 = nc.dram_tensor("bo8", [D], F32R, kind="ExternalInput")
    out = nc.dram_tensor("out", [B, S, D], F32, kind="ExternalOutput")

    with tile.TileContext(nc) as tc:
        with (
            tc.tile_pool(name="const", bufs=1) as cpool,
            tc.tile_pool(name="xtp", bufs=1) as xt_pool,
            tc.tile_pool(name="qkv", bufs=2) as qkv_pool,
            tc.tile_pool(name="otp", bufs=1) as ot_pool,
            tc.tile_pool(name="nrm", bufs=2) as nrm_pool,
            tc.tile_pool(name="ep", bufs=4) as e_pool,
            tc.tile_pool(name="osb", bufs=2) as osb_pool,
            tc.tile_pool(name="ps1", bufs=2, space="PSUM") as ps1,
            tc.tile_pool(name="ps2", bufs=1, space="PSUM") as ps2,
            tc.tile_pool(name="ps3", bufs=2, space="PSUM") as ps3,
        ):
            # ---- constants ----
            wq_sb = cpool.tile([P, DC, HH], F32R, tag="wq", name="wq_sb")
            wk_sb = cpool.tile([P, DC, HH], F32R, tag="wk", name="wk_sb")
            wv_sb = cpool.tile([P, DC, HH], F32R, tag="wv", name="wv_sb")
            wo_sb = cpool.tile([P, D], F32R, tag="wo", name="wo_sb")
            nc.sync.dma_start(wq_sb[:], wq.rearrange("(o p) f -> p o f", p=P))
            nc.sync.dma_start(wk_sb[:], wk.rearrange("(o p) f -> p o f", p=P))
            nc.sync.dma_start(wv_sb[:], wv.rearrange("(o p) f -> p o f", p=P))
            nc.sync.dma_start(wo_sb[:], wo[:])
            ones_sb = cpool.tile([P, P], F32R, tag="ones", name="ones_sb")
            nc.vector.memset(ones_sb[:].bitcast(F32), 1.0)
            ident_sb = cpool.tile([P, P], F32, tag="ident", name="ident_sb")
            make_identity(nc, ident_sb[:])
            if with_bias_qkv:
                # per-partition bias columns: [:, 0]=bq, [:, 1]=bk, [:, 2]=bv
                bqkvt_sb = cpool.tile([HH, 3], F32, tag="bqkvt", name="bqkvt_sb")
                for i in range(3):
                    nc.sync.dma_start(
                        bqkvt_sb[:, i:i + 1], bqkv[i:i + 1, :].rearrange("a f -> f a")
                    )
            if with_bias_o:
                bo8_sb = cpool.tile([1, D], F32R, tag="bo8", name="bo8_sb")
                nc.sync.dma_start(bo8_sb[:], bo8.rearrange("(a d) -> a d", a=1))

            # per-batch state
            st = [dict() for _ in range(B)]

            def load_xt(b):
                xt_sb = xt_pool.tile([P, DC, S], F32R, tag="xt", name="xt_sb")
                for d in range(DC):
                    nc.sync.dma_start(xt_sb[:, d, :], xt[b, d * P:(d + 1) * P, :])
                st[b]["xt"] = xt_sb

            def alloc_qkv(b):
                st[b]["qt"] = qkv_pool.tile([P, QJ, NQ], F32R, tag="qt", name="qt_sb")
                st[b]["kt"] = qkv_pool.tile([P, QJ, NQ], F32R, tag="kt", name="kt_sb")
                st[b]["vt"] = qkv_pool.tile([P, QJ, NQ], F32, tag="vt", name="vt_sb",
                                            bufs=1)
                v_sb = qkv_pool.tile([P, KC, HPC, HD + 1], F32R, tag="v", name="v_sb")
                nc.vector.memset(v_sb[:, :, :, HD:].bitcast(F32), 1.0)
                st[b]["v"] = v_sb

            def proj_copy(b, w_idx, dst, pps, half):
                if with_bias_qkv:
                    nc.scalar.activation(
                        dst[:, half * 2:(half + 1) * 2, :], pps[:],
                        AF.Identity, bias=bqkvt_sb[:, w_idx:w_idx + 1],
                    )
                else:
                    nc.scalar.copy(dst[:, half * 2:(half + 1) * 2, :], pps[:])

            def proj_b0_tensor(b, w_idx):
                # d-major: both halves' psum tiles live, matmuls grouped by
                # d-chunk so they can start as each X^T chunk lands
                w_sb = (wq_sb, wk_sb, wv_sb)[w_idx]
                dst = (st[b]["qt"], st[b]["kt"], st[b]["vt"])[w_idx]
                pps = [ps1.tile([P, 2, NQ], F32, tag="st", name=f"proj_ps{h}")
                       for h in range(2)]
                for d in range(DC):
                    for half in range(2):
                        for j2 in range(2):
                            nc.tensor.matmul(
                                pps[half][:, j2, :],
                                lhsT=w_sb[:, d, :],
                                rhs=st[b]["xt"][:, d, (half * 2 + j2) * NQ:(half * 2 + j2 + 1) * NQ],
                                start=(d == 0), stop=(d == DC - 1),
                            )
                for half in range(2):
                    proj_copy(b, w_idx, dst, pps[half], half)

            def proj_b1_packet(b, w_idx, half):
                # half-major packet from ps2 (used while b0's attention runs)
                w_sb = (wq_sb, wk_sb, wv_sb)[w_idx]
                dst = (st[b]["qt"], st[b]["kt"], st[b]["vt"])[w_idx]
                pps = ps2.tile([P, 2, NQ], F32, tag="op", name="proj_ps")
                for j2 in range(2):
                    for d in range(DC):
                        nc.tensor.matmul(
                            pps[:, j2, :],
                            lhsT=w_sb[:, d, :],
                            rhs=st[b]["xt"][:, d, (half * 2 + j2) * NQ:(half * 2 + j2 + 1) * NQ],
                            start=(d == 0), stop=(d == DC - 1),
                        )
                proj_copy(b, w_idx, dst, pps, half)

            def v_transpose(b, sc):
                tp = ps3.tile([P, P], F32, tag="o", name="tr_ps")
                nc.tensor.transpose(
                    tp[:], st[b]["vt"][:, sc // 4, (sc % 4) * P:(sc % 4 + 1) * P],
                    ident_sb[:],
                )
                for h in range(HPC):
                    nc.vector.tensor_copy(
                        st[b]["v"][:, sc, h, :HD], tp[:, h * HD:(h + 1) * HD]
                    )

            def attn_alloc(b):
                st[b]["ot"] = ot_pool.tile([P, QJ, NQ], F32, tag=f"ot{b}",
                                           name="ot_sb")
                st[b]["otn"] = ot_pool.tile([P, QJ, NQ], F32R, tag=f"otn{b}",
                                            name="otn_sb")

            def attn_qj(b, qj, pump):
                """score/exp/attnV for both heads, interleaved group-by-group."""
                qt, kt, v = st[b]["qt"], st[b]["kt"], st[b]["v"]
                ngroups = (2 * (qj + 1)) if causal else (KC // GK)
                o_ps = [ps3.tile([HD + 1, NQ], F32, tag="o", name=f"o_ps{h}")
                        for h in range(HPC)]
                for g in range(ngroups):
                    stp = []
                    for h in range(HPC):
                        h0 = h * HD
                        sp = ps1.tile([P, GK, NQ], F32, tag="st", name=f"st_ps{h}")
                        stp.append(sp)
                        for c2 in range(GK):
                            ki = g * GK + c2
                            nc.tensor.matmul(
                                sp[:, c2, :],
                                lhsT=kt[h0:h0 + HD, ki // 4, (ki % 4) * P:(ki % 4 + 1) * P],
                                rhs=qt[h0:h0 + HD, qj, :],
                                start=True, stop=True,
                            )
                    for h in range(HPC):
                        e_sb = e_pool.tile([P, GK, NQ], F32R, tag="e", name="e_sb")
                        nc.scalar.activation(e_sb[:], stp[h][:], AF.Exp, scale=0.125)
                        if causal and g >= 2 * qj:
                            nc.gpsimd.affine_select(
                                out=e_sb[:], in_=e_sb[:],
                                compare_op=mybir.AluOpType.is_ge, fill=0.0,
                                base=qj * NQ - g * GK * P,
                                pattern=[[-P, GK], [1, NQ]],
                                channel_multiplier=-1,
                            )
                        for c2 in range(GK):
                            ki = g * GK + c2
                            nc.tensor.matmul(
                                o_ps[h][:],
                                lhsT=v[:, ki, h, :],
                                rhs=e_sb[:, c2, :],
                                start=(g == 0 and c2 == 0),
                                stop=(g == ngroups - 1 and c2 == GK - 1),
                            )
                    pump(b, qj, g)
                return o_ps

            def norm_outproj_qj(b, qj, o_ps):
                ot, otn = st[b]["ot"], st[b]["otn"]
                # collect the two heads' denominators at partitions 0 / 32
                rsq = nrm_pool.tile([33, NQ], F32, tag="rsq", name="rsq")
                for h in range(HPC):
                    nc.vector.tensor_copy(
                        rsq[32 * h:32 * h + 1, :], o_ps[h][HD:HD + 1, :]
                    )
                    nc.vector.tensor_copy(
                        ot[h * HD:(h + 1) * HD, qj, :], o_ps[h][:HD, :]
                    )
                rscq = nrm_pool.tile([33, NQ], F32, tag="rscq", name="rscq")
                rinq = nrm_pool.tile([33, NQ], F32R, tag="rinq", name="rinq")
                nc.vector.reciprocal_approx_fast(out=rscq[:], in_=rsq[:])
                nc.vector._custom_dve(
                    RECIPROCAL_APPROX_NR, out=rinq[:], in0=rsq[:], in1=rscq[:],
                    s0=2.0,
                )
                for h in range(HPC):
                    h0 = h * HD
                    bc_ps = ps3.tile([HD, NQ], F32, tag="o", name="bc_ps")
                    nc.tensor.matmul(
                        bc_ps[:],
                        lhsT=ones_sb[32 * h:32 * h + 1, :HD],
                        rhs=rinq[32 * h:32 * h + 1, :],
                        start=True, stop=True,
                    )
                    nc.vector.tensor_mul(
                        otn[h0:h0 + HD, qj, :], ot[h0:h0 + HD, qj, :], bc_ps[:]
                    )
                # output projection for this q-tile's 4 s-chunks
                for sc4 in range(4):
                    sc = qj * 4 + sc4
                    ops = ps2.tile([P, 2, NQ], F32, tag="op", name="op_ps")
                    for fc in range(2):
                        if with_bias_o:
                            nc.tensor.matmul(
                                ops[:, fc, :], lhsT=ones_sb[0:1, :P],
                                rhs=bo8_sb[:, fc * NQ:(fc + 1) * NQ],
                                start=True, stop=False,
                            )
                        nc.tensor.matmul(
                            ops[:, fc, :],
                            lhsT=otn[:, qj, sc4 * P:(sc4 + 1) * P],
                            rhs=wo_sb[:, fc * NQ:(fc + 1) * NQ],
                            start=not with_bias_o, stop=True,
                        )
                    out_sb = osb_pool.tile([P, 2, NQ], F32, tag="out", name="out_sb")
                    if sc % 2 == 0:
                        nc.scalar.copy(out_sb[:], ops[:])
                    else:
                        nc.vector.tensor_copy(out_sb[:], ops[:])
                    nc.sync.dma_start(
                        out[b, sc * P:(sc + 1) * P, :],
                        out_sb.rearrange("p a n -> p (a n)"),
                    )

            # ---------------- program ----------------
            load_xt(0)
            alloc_qkv(0)
            for w_idx in range(3):
                proj_b0_tensor(0, w_idx)
            for sc in range(KC):
                v_transpose(0, sc)

            # batch-1 prep work, deferred into batch-0's attention stream
            load_xt(1)          # DMA waits for xt slot release (end of b0 proj)
            alloc_qkv(1)
            deferred = []
            for w_idx in range(3):
                for half in range(2):
                    deferred.append(
                        (lambda w=w_idx, hf=half: proj_b1_packet(1, w, hf))
                    )
            for sc in range(KC):
                deferred.append(lambda s=sc: v_transpose(1, s))

            def pump_b0(b, qj, g):
                if qj >= 2 and deferred:
                    deferred.pop(0)()

            def pump_none(b, qj, g):
                pass

            attn_alloc(0)
            for qj in range(QJ):
                o_ps = attn_qj(0, qj, pump_b0)
                norm_outproj_qj(0, qj, o_ps)

            while deferred:
                deferred.pop(0)()

            attn_alloc(1)
            for qj in range(QJ):
                o_ps = attn_qj(1, qj, pump_none)
                norm_outproj_qj(1, qj, o_ps)

    nc.compile()
    _NC_CACHE[key] = nc
    return nc


def _check_causal(mask: np.ndarray) -> bool:
    m = np.asarray(mask).reshape(mask.shape[-2], mask.shape[-1])
    s = m.shape[0]
    if np.array_equal(m, np.tril(np.ones((s, s), dtype=bool))):
        return True
    if m.all():
        return False
    raise NotImplementedError("only causal or all-true masks are supported")


def kernel(inputs_q, mask, Wq, bq, Wk, bk, Wv, bv, Wo, bo, _trace=False,
           _trace_cores=None):
    inputs_q = np.asarray(inputs_q, dtype=np.float32)
    Wq = np.asarray(Wq, dtype=np.float32).reshape(D, H * HD)
    Wk = np.asarray(Wk, dtype=np.float32).reshape(D, H * HD)
    Wv = np.asarray(Wv, dtype=np.float32).reshape(D, H * HD)
    Wo = np.asarray(Wo, dtype=np.float32).reshape(H * HD, D)
    bq = np.asarray(bq, dtype=np.float32).reshape(H * HD)
    bk = np.asarray(bk, dtype=np.float32).reshape(H * HD)
    bv = np.asarray(bv, dtype=np.float32).reshape(H * HD)
    bo = np.asarray(bo, dtype=np.float32).reshape(D)

    causal = _check_causal(mask)
    with_bias_qkv = bool(bq.any() or bk.any() or bv.any())
    with_bias_o = bool(bo.any())

    nc = _build_nc(with_bias_qkv, with_bias_o, causal)

    xt = np.ascontiguousarray(inputs_q.transpose(0, 2, 1))  # [B, D, S]
    in_maps = []
    for c in range(NCORES):
        f0, f1 = c * HH, (c + 1) * HH
        m = {
            "xt": xt,
            "wq": np.ascontiguousarray(Wq[:, f0:f1]),
            "wk": np.ascontiguousarray(Wk[:, f0:f1]),
            "wv": np.ascontiguousarray(Wv[:, f0:f1]),
            "wo": np.ascontiguousarray(Wo[f0:f1, :]),
        }
        if with_bias_qkv:
            m["bqkv"] = np.ascontiguousarray(
                np.stack([bq[f0:f1], bk[f0:f1], bv[f0:f1]])
            )
        if with_bias_o:
            m["bo8"] = np.ascontiguousarray(bo / NCORES)
        in_maps.append(m)

    kwargs = {}
    if _trace:
        kwargs["trace"] = True
        if _trace_cores is not None:
            kwargs["trace_cores"] = _trace_cores
    res = run_bass_kernel_spmd(nc, in_maps, core_ids=list(range(NCORES)), **kwargs)

    acc = np.zeros((B, S, D), dtype=np.float64)
    for c in range(NCORES):
        acc += res.results[c]["out"]
    if not with_bias_o:
        acc += bo  # bo is zero here, but keep the math explicit
    out = acc.astype(np.float32)
    if _trace:
        return out, res
    return out
